# revision 28
# baseline (speedup 1.0000x reference)
"""MDCT kernel for Trainium2 (8 NeuronCores, batch-parallel), folded DCT-IV form.

Math: frame f (hop N=1024, len 2N, center-padded) folds to an N-vector u and
out[f] = DCT-IV(u).  With x2 = x.reshape(1024, 1024) and y1 = w[:N]*x2[r],
y2 = w[N:]*x2[r] (per-row windowing):
    u[f, m]      = -y2[f, 511-m] - y2[f, 512+m]      (m < 512,  row f)
    u[f, 512+p]  =  y1[f-1, p]   - y1[f-1, 1023-p]   (p < 512,  row f-1)
so each x2 row r yields uLo[r] (frame r) and uHi[r] (frame r+1), and
    out[f, k] = sum_m u[f, m] * D4[m, k],   D4 = sqrt(2/N) DCT-IV matrix.

This halves the matmul contraction (1024 vs 2048) vs the direct form.  The
fold runs on the vector engine (reversals are negative-stride APs), u is
transposed on the PE in bf16 (1 cyc/row), and the DCT matmuls run in bf16
(same PE rate as fp32r, but FWL-fast weight loads and half the
weight-matrix HBM traffic).
"""

import numpy as np
import ml_dtypes

import concourse.bass as bass
import concourse.bacc as bacc
import concourse.mybir as mybir
import concourse.tile as tile
from concourse import masks
from concourse.bass_utils import run_bass_kernel_spmd

B = 8
T = 1 << 20
R = 1024          # rows of x2 per channel (T // hop)
CN = 1024         # row width (hop) = N
NF = 1025         # output frames
NK = 1024         # output bins
F32 = mybir.dt.float32
BF16 = mybir.dt.bfloat16

_NC_CACHE = None
_CONST_CACHE = None


def build_nc() -> bass.Bass:
    nc = bacc.Bacc("TRN2", target_bir_lowering=False, debug=False)
    x = nc.dram_tensor("x", [R, CN], F32, kind="ExternalInput").ap()
    w1r = nc.dram_tensor("w1r", [128, CN], BF16, kind="ExternalInput").ap()
    w2nr = nc.dram_tensor("w2nr", [128, CN], BF16, kind="ExternalInput").ap()
    d4 = nc.dram_tensor("d4", [8, 128, NK], BF16, kind="ExternalInput").ap()
    out = nc.dram_tensor("out", [NF, NK], F32, kind="ExternalOutput").ap()

    with tile.TileContext(nc) as tc:
        with (
            tc.tile_pool(name="persist", bufs=1) as persist,
            tc.tile_pool(name="xin", bufs=1) as xin,
            tc.tile_pool(name="ypool", bufs=6) as ypool,
            tc.tile_pool(name="upool", bufs=4) as upool,
            tc.tile_pool(name="outp", bufs=4) as outp,
            tc.tile_pool(name="tps", bufs=4, space="PSUM") as tps,
            tc.tile_pool(name="mmps", bufs=4, space="PSUM") as mmps,
        ):
            w1 = persist.tile([128, CN], BF16)
            w2n = persist.tile([128, CN], BF16)

            ident = persist.tile([128, 128], BF16)
            masks.make_identity(nc, ident[:])

            dt = persist.tile([128, 8, NK], BF16)
            ulot = persist.tile([128, 4, R], BF16)
            uhit = persist.tile([128, 4, NF], BF16)
            nc.vector.memset(uhit[:, :, 0:1], 0.0)

            xts = [xin.tile([128, CN], F32, name=f"xt{i}") for i in range(8)]

            def dma_x(r):
                nc.sync.dma_start(xts[r][:], x[r * 128:(r + 1) * 128, :])

            dma_x(0)
            nc.sync.dma_start(w2n[:], w2nr)
            nc.sync.dma_start(w1[:], w1r)
            dma_x(1)
            for ci in range(8):
                nc.sync.dma_start(dt[:, ci, :], d4[ci])
            for r in range(2, 8):
                dma_x(r)

            warm = tps.tile([128, 512], BF16, tag="tp")
            for _ in range(115):
                nc.tensor.transpose(warm[:, 0:128], ident[:], ident[:])

            uns = [None] * 8

            def fold(r: int):
                xt = xts[r]
                un = upool.tile([128, CN], BF16)
                y2n = ypool.tile([128, CN], BF16, tag="y2n")
                nc.vector.tensor_tensor(y2n[:], xt[:], w2n[:], mybir.AluOpType.mult)
                # uLo[m] = y2n[511-m] + y2n[512+m]   (y2n = -w2*x)
                nc.vector.tensor_tensor(
                    un[:, 0:512], y2n[:, 511::-1], y2n[:, 512:1024],
                    mybir.AluOpType.add,
                )
                y1 = ypool.tile([128, CN], BF16, tag="y1")
                nc.vector.tensor_tensor(y1[:], xt[:], w1[:], mybir.AluOpType.mult)
                # uHi[p] = y1[p] - y1[1023-p]
                nc.vector.tensor_tensor(
                    un[:, 512:1024], y1[:, 0:512], y1[:, 1023:511:-1],
                    mybir.AluOpType.subtract,
                )
                uns[r] = un

            def transpose_u(r: int):
                un = uns[r]
                r0 = r * 128
                plo = tps.tile([128, 512], BF16, tag="tp")
                for ci in range(4):
                    nc.tensor.transpose(
                        plo[:, ci * 128:(ci + 1) * 128],
                        un[:, ci * 128:(ci + 1) * 128], ident[:],
                    )
                nc.vector.tensor_copy(ulot[:, 0:4, r0:r0 + 128], plo[:])
                phi = tps.tile([128, 512], BF16, tag="tp")
                for ci in range(4):
                    nc.tensor.transpose(
                        phi[:, ci * 128:(ci + 1) * 128],
                        un[:, 512 + ci * 128:512 + (ci + 1) * 128], ident[:],
                    )
                nc.scalar.copy(uhit[:, 0:4, 1 + r0:1 + r0 + 128], phi[:])

            def wslice(ci, f0):
                if ci < 4:
                    return ulot[:, ci, f0:f0 + 128]
                return uhit[:, ci - 4, f0:f0 + 128]

            def last_frame():
                # f=1024: only the uHi half (row 1023) contributes.
                pa = mmps.tile([1, 512], F32, tag="mm")
                pb = mmps.tile([1, 512], F32, tag="mm")
                for ci in range(4):
                    wsl = uhit[:, ci, 1024:1025]
                    nc.tensor.matmul(
                        pa[:], wsl, dt[:, 4 + ci, 0:512],
                        start=(ci == 0), stop=(ci == 3),
                    )
                    nc.tensor.matmul(
                        pb[:], wsl, dt[:, 4 + ci, 512:1024],
                        start=(ci == 0), stop=(ci == 3),
                    )
                ot = outp.tile([1, NK], F32, tag="ot_last")
                nc.scalar.copy(ot[:, 0:512], pa[:])
                nc.scalar.copy(ot[:, 512:1024], pb[:])
                nc.sync.dma_start(out[1024:1025, :], ot[:])

            def dct_tile(j: int):
                f0 = j * 128
                ot = outp.tile([128, NK], F32)
                pa = mmps.tile([128, 512], F32, tag="mm")
                for ci in range(8):
                    nc.tensor.matmul(
                        pa[:], wslice(ci, f0), dt[:, ci, 0:512],
                        start=(ci == 0), stop=(ci == 7),
                    )
                nc.scalar.copy(ot[:, 0:512], pa[:])
                nc.sync.dma_start(out[f0:f0 + 128, 0:512], ot[:, 0:512])
                pb = mmps.tile([128, 512], F32, tag="mm")
                for ci in range(8):
                    nc.tensor.matmul(
                        pb[:], wslice(ci, f0), dt[:, ci, 512:1024],
                        start=(ci == 0), stop=(ci == 7),
                    )
                nc.scalar.copy(ot[:, 512:1024], pb[:])
                nc.sync.dma_start(out[f0:f0 + 128, 512:1024], ot[:, 512:1024])

            fold(0)
            transpose_u(0)
            fold(1)
            for r in range(8):
                if r + 2 < 8:
                    fold(r + 2)
                if r + 1 < 8:
                    transpose_u(r + 1)
                if r == 7:
                    last_frame()
                dct_tile(r)

    return nc


def make_consts(window: np.ndarray):
    w = window.astype(np.float64)
    w1r = np.broadcast_to(w[:CN].astype(ml_dtypes.bfloat16), (128, CN)).copy()
    w2nr = np.broadcast_to((-w[CN:]).astype(ml_dtypes.bfloat16), (128, CN)).copy()
    m = np.arange(NK, dtype=np.float64)[:, None]
    k = np.arange(NK, dtype=np.float64)[None, :]
    d = (np.sqrt(2.0 / NK) * np.cos(np.pi / NK * (m + 0.5) * (k + 0.5)))
    d4 = d.astype(ml_dtypes.bfloat16).reshape(8, 128, NK)
    return w1r, w2nr, d4


def _get_nc() -> bass.Bass:
    global _NC_CACHE
    if _NC_CACHE is None:
        _NC_CACHE = build_nc()
        _NC_CACHE.compile()
    return _NC_CACHE


def run_spmd(x: np.ndarray, window: np.ndarray, **kwargs):
    """Shard, run on 8 cores, return (stacked output, BassKernelResults)."""
    global _CONST_CACHE
    if _CONST_CACHE is None or _CONST_CACHE[0] != window.tobytes():
        _CONST_CACHE = (window.tobytes(), make_consts(window))
    w1r, w2nr, d4 = _CONST_CACHE[1]
    in_maps = [
        {"x": np.ascontiguousarray(x[b].reshape(R, CN)),
         "w1r": w1r, "w2nr": w2nr, "d4": d4}
        for b in range(B)
    ]
    res = run_bass_kernel_spmd(nc=_get_nc(), in_maps=in_maps,
                               core_ids=list(range(B)), **kwargs)
    out = np.stack([res.results[b]["out"] for b in range(B)], axis=0)
    return out, res


def kernel(x: np.ndarray, window: np.ndarray) -> np.ndarray:
    out, _ = run_spmd(np.asarray(x), np.asarray(window))
    return out


# revision 29
# speedup vs baseline: 1.0649x; 1.0649x over previous
"""MDCT kernel for Trainium2 (8 NeuronCores, batch-parallel), folded DCT-IV form.

Math: frame f (hop N=1024, len 2N, center-padded) folds to an N-vector u and
out[f] = DCT-IV(u).  With x2 = x.reshape(1024, 1024) and y1 = w[:N]*x2[r],
y2 = w[N:]*x2[r] (per-row windowing):
    u[f, m]      = -y2[f, 511-m] - y2[f, 512+m]      (m < 512,  row f)
    u[f, 512+p]  =  y1[f-1, p]   - y1[f-1, 1023-p]   (p < 512,  row f-1)
so each x2 row r yields uLo[r] (frame r) and uHi[r] (frame r+1), and
    out[f, k] = sum_m u[f, m] * D4[m, k],   D4 = sqrt(2/N) DCT-IV matrix.

This halves the matmul contraction (1024 vs 2048) vs the direct form.  The
fold runs on the vector engine (reversals are negative-stride APs), u is
transposed on the PE in bf16 (1 cyc/row), and the DCT matmuls run in bf16
(same PE rate as fp32r, but FWL-fast weight loads and half the
weight-matrix HBM traffic).
"""

import numpy as np
import ml_dtypes

import concourse.bass as bass
import concourse.bacc as bacc
import concourse.mybir as mybir
import concourse.tile as tile
from concourse import masks
from concourse.bass_utils import run_bass_kernel_spmd

B = 8
T = 1 << 20
R = 1024          # rows of x2 per channel (T // hop)
CN = 1024         # row width (hop) = N
NF = 1025         # output frames
NK = 1024         # output bins
F32 = mybir.dt.float32
BF16 = mybir.dt.bfloat16

_NC_CACHE = None
_CONST_CACHE = None


def build_nc() -> bass.Bass:
    nc = bacc.Bacc("TRN2", target_bir_lowering=False, debug=False)
    x = nc.dram_tensor("x", [R, CN], F32, kind="ExternalInput").ap()
    w1r = nc.dram_tensor("w1r", [128, CN], BF16, kind="ExternalInput").ap()
    w2nr = nc.dram_tensor("w2nr", [128, CN], BF16, kind="ExternalInput").ap()
    d4 = nc.dram_tensor("d4", [8, 128, NK], BF16, kind="ExternalInput").ap()
    out = nc.dram_tensor("out", [NF, NK], F32, kind="ExternalOutput").ap()

    with tile.TileContext(nc) as tc:
        with (
            tc.tile_pool(name="persist", bufs=1) as persist,
            tc.tile_pool(name="xin", bufs=1) as xin,
            tc.tile_pool(name="ypool", bufs=6) as ypool,
            tc.tile_pool(name="upool", bufs=4) as upool,
            tc.tile_pool(name="outp", bufs=4) as outp,
            tc.tile_pool(name="tps", bufs=4, space="PSUM") as tps,
            tc.tile_pool(name="mmps", bufs=4, space="PSUM") as mmps,
        ):
            w1 = persist.tile([128, CN], BF16)
            w2n = persist.tile([128, CN], BF16)

            ident = persist.tile([128, 128], BF16)
            masks.make_identity(nc, ident[:])

            dt = persist.tile([128, 8, NK], BF16)
            ulot = persist.tile([128, 4, R], BF16)
            uhit = persist.tile([128, 4, NF], BF16)
            nc.vector.memset(uhit[:, :, 0:1], 0.0)

            xts = [xin.tile([128, CN], F32, name=f"xt{i}") for i in range(8)]

            def dma_x(r):
                nc.sync.dma_start(xts[r][:], x[r * 128:(r + 1) * 128, :])

            dma_x(0)
            nc.sync.dma_start(w2n[:], w2nr)
            nc.sync.dma_start(w1[:], w1r)
            dma_x(1)
            for ci in (0, 1, 2, 3):
                nc.sync.dma_start(dt[:, ci, :], d4[ci])
            dma_x(2)
            for ci in (4, 5, 6, 7):
                nc.sync.dma_start(dt[:, ci, :], d4[ci])
            for r in range(3, 8):
                dma_x(r)

            warm = tps.tile([128, 512], BF16, tag="tp")
            for _ in range(115):
                nc.tensor.transpose(warm[:, 0:128], ident[:], ident[:])

            uns = [None] * 8

            def fold(r: int):
                xt = xts[r]
                un = upool.tile([128, CN], BF16)
                y2n = ypool.tile([128, CN], BF16, tag="y2n")
                nc.vector.tensor_tensor(y2n[:], xt[:], w2n[:], mybir.AluOpType.mult)
                # uLo[m] = y2n[511-m] + y2n[512+m]   (y2n = -w2*x)
                nc.vector.tensor_tensor(
                    un[:, 0:512], y2n[:, 511::-1], y2n[:, 512:1024],
                    mybir.AluOpType.add,
                )
                y1 = ypool.tile([128, CN], BF16, tag="y1")
                nc.vector.tensor_tensor(y1[:], xt[:], w1[:], mybir.AluOpType.mult)
                # uHi[p] = y1[p] - y1[1023-p]
                nc.vector.tensor_tensor(
                    un[:, 512:1024], y1[:, 0:512], y1[:, 1023:511:-1],
                    mybir.AluOpType.subtract,
                )
                uns[r] = un

            def transpose_u(r: int):
                un = uns[r]
                r0 = r * 128
                plo = tps.tile([128, 512], BF16, tag="tp")
                for ci in range(4):
                    nc.tensor.transpose(
                        plo[:, ci * 128:(ci + 1) * 128],
                        un[:, ci * 128:(ci + 1) * 128], ident[:],
                    )
                nc.vector.tensor_copy(ulot[:, 0:4, r0:r0 + 128], plo[:])
                phi = tps.tile([128, 512], BF16, tag="tp")
                for ci in range(4):
                    nc.tensor.transpose(
                        phi[:, ci * 128:(ci + 1) * 128],
                        un[:, 512 + ci * 128:512 + (ci + 1) * 128], ident[:],
                    )
                nc.scalar.copy(uhit[:, 0:4, 1 + r0:1 + r0 + 128], phi[:])

            def wslice(ci, f0):
                if ci < 4:
                    return ulot[:, ci, f0:f0 + 128]
                return uhit[:, ci - 4, f0:f0 + 128]

            def last_frame():
                # f=1024: only the uHi half (row 1023) contributes.
                pa = mmps.tile([1, 512], F32, tag="mm")
                pb = mmps.tile([1, 512], F32, tag="mm")
                for ci in range(4):
                    wsl = uhit[:, ci, 1024:1025]
                    nc.tensor.matmul(
                        pa[:], wsl, dt[:, 4 + ci, 0:512],
                        start=(ci == 0), stop=(ci == 3),
                    )
                    nc.tensor.matmul(
                        pb[:], wsl, dt[:, 4 + ci, 512:1024],
                        start=(ci == 0), stop=(ci == 3),
                    )
                ot = outp.tile([1, NK], F32, tag="ot_last")
                nc.scalar.copy(ot[:, 0:512], pa[:])
                nc.scalar.copy(ot[:, 512:1024], pb[:])
                nc.sync.dma_start(out[1024:1025, :], ot[:])

            def dct_tile(j: int):
                f0 = j * 128
                ot = outp.tile([128, NK], F32)
                pa = mmps.tile([128, 512], F32, tag="mm")
                for ci in range(8):
                    nc.tensor.matmul(
                        pa[:], wslice(ci, f0), dt[:, ci, 0:512],
                        start=(ci == 0), stop=(ci == 7),
                    )
                nc.scalar.copy(ot[:, 0:512], pa[:])
                nc.sync.dma_start(out[f0:f0 + 128, 0:512], ot[:, 0:512])
                pb = mmps.tile([128, 512], F32, tag="mm")
                for ci in range(8):
                    nc.tensor.matmul(
                        pb[:], wslice(ci, f0), dt[:, ci, 512:1024],
                        start=(ci == 0), stop=(ci == 7),
                    )
                nc.scalar.copy(ot[:, 512:1024], pb[:])
                nc.sync.dma_start(out[f0:f0 + 128, 512:1024], ot[:, 512:1024])

            fold(0)
            transpose_u(0)
            fold(1)
            for r in range(8):
                if r + 2 < 8:
                    fold(r + 2)
                if r + 1 < 8:
                    transpose_u(r + 1)
                if r == 7:
                    last_frame()
                dct_tile(r)

    return nc


def make_consts(window: np.ndarray):
    w = window.astype(np.float64)
    w1r = np.broadcast_to(w[:CN].astype(ml_dtypes.bfloat16), (128, CN)).copy()
    w2nr = np.broadcast_to((-w[CN:]).astype(ml_dtypes.bfloat16), (128, CN)).copy()
    m = np.arange(NK, dtype=np.float64)[:, None]
    k = np.arange(NK, dtype=np.float64)[None, :]
    d = (np.sqrt(2.0 / NK) * np.cos(np.pi / NK * (m + 0.5) * (k + 0.5)))
    d4 = d.astype(ml_dtypes.bfloat16).reshape(8, 128, NK)
    return w1r, w2nr, d4


def _get_nc() -> bass.Bass:
    global _NC_CACHE
    if _NC_CACHE is None:
        _NC_CACHE = build_nc()
        _NC_CACHE.compile()
    return _NC_CACHE


def run_spmd(x: np.ndarray, window: np.ndarray, **kwargs):
    """Shard, run on 8 cores, return (stacked output, BassKernelResults)."""
    global _CONST_CACHE
    if _CONST_CACHE is None or _CONST_CACHE[0] != window.tobytes():
        _CONST_CACHE = (window.tobytes(), make_consts(window))
    w1r, w2nr, d4 = _CONST_CACHE[1]
    in_maps = [
        {"x": np.ascontiguousarray(x[b].reshape(R, CN)),
         "w1r": w1r, "w2nr": w2nr, "d4": d4}
        for b in range(B)
    ]
    res = run_bass_kernel_spmd(nc=_get_nc(), in_maps=in_maps,
                               core_ids=list(range(B)), **kwargs)
    out = np.stack([res.results[b]["out"] for b in range(B)], axis=0)
    return out, res


def kernel(x: np.ndarray, window: np.ndarray) -> np.ndarray:
    out, _ = run_spmd(np.asarray(x), np.asarray(window))
    return out


# revision 30
# speedup vs baseline: 1.0684x; 1.0033x over previous
"""MDCT kernel for Trainium2 (8 NeuronCores, batch-parallel), folded DCT-IV form.

Math: frame f (hop N=1024, len 2N, center-padded) folds to an N-vector u and
out[f] = DCT-IV(u).  With x2 = x.reshape(1024, 1024) and y1 = w[:N]*x2[r],
y2 = w[N:]*x2[r] (per-row windowing):
    u[f, m]      = -y2[f, 511-m] - y2[f, 512+m]      (m < 512,  row f)
    u[f, 512+p]  =  y1[f-1, p]   - y1[f-1, 1023-p]   (p < 512,  row f-1)
so each x2 row r yields uLo[r] (frame r) and uHi[r] (frame r+1), and
    out[f, k] = sum_m u[f, m] * D4[m, k],   D4 = sqrt(2/N) DCT-IV matrix.

This halves the matmul contraction (1024 vs 2048) vs the direct form.  The
fold runs on the vector engine (reversals are negative-stride APs), u is
transposed on the PE in bf16 (1 cyc/row), and the DCT matmuls run in bf16
(same PE rate as fp32r, but half the weight-matrix HBM traffic).

Schedule notes (measured on HW):
- warm PE: MM N=512 ~216 ns, transpose ~56-107 ns; a run of dummy ident
  transposes spans the DMA-fill phase so the HAM clock gate is at 2.4 GHz
  when real work starts (a >3.4 us PE idle re-throttles to 1.2 GHz).
- emission is software-pipelined (fold r+2 / transpose r+1 / DCT r) so no
  engine queue blocks on a consumer stage; PSUM->SBUF u copies split
  between DVE (uLo) and ACT (uHi), out copies on ACT.
- DMA issue order: x0, w, x1, D0-3, x2, D4-7, x3..x7 - the fill phase is
  HBM-bandwidth-bound and DCT tile 0 needs all 8 D chunks early, while
  folds for rows 3+ happen well after their x tiles land.
- dma_start_transpose (XBAR) was 1.24 us per 128x128 call on the Sync
  sequencer (79 us total) - don't use it here; PE transposes are ~20x
  cheaper in stream.
"""

import numpy as np
import ml_dtypes

import concourse.bass as bass
import concourse.bacc as bacc
import concourse.mybir as mybir
import concourse.tile as tile
from concourse import masks
from concourse.bass_utils import run_bass_kernel_spmd

B = 8
T = 1 << 20
R = 1024          # rows of x2 per channel (T // hop)
CN = 1024         # row width (hop) = N
NF = 1025         # output frames
NK = 1024         # output bins
F32 = mybir.dt.float32
BF16 = mybir.dt.bfloat16

_NC_CACHE = None
_CONST_CACHE = None


def build_nc() -> bass.Bass:
    nc = bacc.Bacc("TRN2", target_bir_lowering=False, debug=False)
    x = nc.dram_tensor("x", [R, CN], F32, kind="ExternalInput").ap()
    w1r = nc.dram_tensor("w1r", [128, CN], BF16, kind="ExternalInput").ap()
    w2nr = nc.dram_tensor("w2nr", [128, CN], BF16, kind="ExternalInput").ap()
    d4 = nc.dram_tensor("d4", [8, 128, NK], BF16, kind="ExternalInput").ap()
    out = nc.dram_tensor("out", [NF, NK], F32, kind="ExternalOutput").ap()

    with tile.TileContext(nc) as tc:
        with (
            tc.tile_pool(name="persist", bufs=1) as persist,
            tc.tile_pool(name="xin", bufs=1) as xin,
            tc.tile_pool(name="ypool", bufs=6) as ypool,
            tc.tile_pool(name="upool", bufs=4) as upool,
            tc.tile_pool(name="outp", bufs=4) as outp,
            tc.tile_pool(name="tps", bufs=4, space="PSUM") as tps,
            tc.tile_pool(name="mmps", bufs=4, space="PSUM") as mmps,
        ):
            w1 = persist.tile([128, CN], BF16)
            w2n = persist.tile([128, CN], BF16)

            ident = persist.tile([128, 128], BF16)
            masks.make_identity(nc, ident[:])

            dt = persist.tile([128, 8, NK], BF16)
            ulot = persist.tile([128, 4, R], BF16)
            uhit = persist.tile([128, 4, NF], BF16)
            nc.vector.memset(uhit[:, :, 0:1], 0.0)

            xts = [xin.tile([128, CN], F32, name=f"xt{i}") for i in range(8)]

            def dma_x(r):
                nc.sync.dma_start(xts[r][:], x[r * 128:(r + 1) * 128, :])

            dma_x(0)
            nc.sync.dma_start(w2n[:], w2nr)
            nc.sync.dma_start(w1[:], w1r)
            dma_x(1)
            for ci in (0, 1, 2, 3):
                nc.sync.dma_start(dt[:, ci, :], d4[ci])
            dma_x(2)
            for ci in (4, 5, 6, 7):
                nc.sync.dma_start(dt[:, ci, :], d4[ci])
            for r in range(3, 8):
                dma_x(r)

            warm = tps.tile([128, 512], BF16, tag="tp")
            for _ in range(115):
                nc.tensor.transpose(warm[:, 0:128], ident[:], ident[:])

            uns = [None] * 8

            def fold(r: int):
                xt = xts[r]
                un = upool.tile([128, CN], BF16)
                y2n = ypool.tile([128, CN], BF16, tag="y2n")
                nc.vector.tensor_tensor(y2n[:], xt[:], w2n[:], mybir.AluOpType.mult)
                # uLo[m] = y2n[511-m] + y2n[512+m]   (y2n = -w2*x)
                nc.vector.tensor_tensor(
                    un[:, 0:512], y2n[:, 511::-1], y2n[:, 512:1024],
                    mybir.AluOpType.add,
                )
                y1 = ypool.tile([128, CN], BF16, tag="y1")
                nc.vector.tensor_tensor(y1[:], xt[:], w1[:], mybir.AluOpType.mult)
                # uHi[p] = y1[p] - y1[1023-p]
                nc.vector.tensor_tensor(
                    un[:, 512:1024], y1[:, 0:512], y1[:, 1023:511:-1],
                    mybir.AluOpType.subtract,
                )
                uns[r] = un

            def transpose_u(r: int):
                un = uns[r]
                r0 = r * 128
                plo = tps.tile([128, 512], BF16, tag="tp")
                for ci in range(4):
                    nc.tensor.transpose(
                        plo[:, ci * 128:(ci + 1) * 128],
                        un[:, ci * 128:(ci + 1) * 128], ident[:],
                    )
                nc.vector.tensor_copy(ulot[:, 0:4, r0:r0 + 128], plo[:])
                phi = tps.tile([128, 512], BF16, tag="tp")
                for ci in range(4):
                    nc.tensor.transpose(
                        phi[:, ci * 128:(ci + 1) * 128],
                        un[:, 512 + ci * 128:512 + (ci + 1) * 128], ident[:],
                    )
                nc.scalar.copy(uhit[:, 0:4, 1 + r0:1 + r0 + 128], phi[:])

            def wslice(ci, f0):
                if ci < 4:
                    return ulot[:, ci, f0:f0 + 128]
                return uhit[:, ci - 4, f0:f0 + 128]

            def last_frame():
                # f=1024: only the uHi half (row 1023) contributes.
                pa = mmps.tile([1, 512], F32, tag="mm")
                pb = mmps.tile([1, 512], F32, tag="mm")
                for ci in range(4):
                    wsl = uhit[:, ci, 1024:1025]
                    nc.tensor.matmul(
                        pa[:], wsl, dt[:, 4 + ci, 0:512],
                        start=(ci == 0), stop=(ci == 3),
                    )
                    nc.tensor.matmul(
                        pb[:], wsl, dt[:, 4 + ci, 512:1024],
                        start=(ci == 0), stop=(ci == 3),
                    )
                ot = outp.tile([1, NK], F32, tag="ot_last")
                nc.scalar.copy(ot[:, 0:512], pa[:])
                nc.scalar.copy(ot[:, 512:1024], pb[:])
                nc.sync.dma_start(out[1024:1025, :], ot[:])

            def dct_tile(j: int):
                f0 = j * 128
                ot = outp.tile([128, NK], F32)
                pa = mmps.tile([128, 512], F32, tag="mm")
                for ci in range(8):
                    nc.tensor.matmul(
                        pa[:], wslice(ci, f0), dt[:, ci, 0:512],
                        start=(ci == 0), stop=(ci == 7),
                    )
                nc.scalar.copy(ot[:, 0:512], pa[:])
                nc.sync.dma_start(out[f0:f0 + 128, 0:512], ot[:, 0:512])
                pb = mmps.tile([128, 512], F32, tag="mm")
                for ci in range(8):
                    nc.tensor.matmul(
                        pb[:], wslice(ci, f0), dt[:, ci, 512:1024],
                        start=(ci == 0), stop=(ci == 7),
                    )
                nc.scalar.copy(ot[:, 512:1024], pb[:])
                nc.sync.dma_start(out[f0:f0 + 128, 512:1024], ot[:, 512:1024])

            fold(0)
            transpose_u(0)
            fold(1)
            for r in range(8):
                if r + 2 < 8:
                    fold(r + 2)
                if r + 1 < 8:
                    transpose_u(r + 1)
                if r == 7:
                    last_frame()
                dct_tile(r)

    return nc


def make_consts(window: np.ndarray):
    w = window.astype(np.float64)
    w1r = np.broadcast_to(w[:CN].astype(ml_dtypes.bfloat16), (128, CN)).copy()
    w2nr = np.broadcast_to((-w[CN:]).astype(ml_dtypes.bfloat16), (128, CN)).copy()
    m = np.arange(NK, dtype=np.float64)[:, None]
    k = np.arange(NK, dtype=np.float64)[None, :]
    d = (np.sqrt(2.0 / NK) * np.cos(np.pi / NK * (m + 0.5) * (k + 0.5)))
    d4 = d.astype(ml_dtypes.bfloat16).reshape(8, 128, NK)
    return w1r, w2nr, d4


def _get_nc() -> bass.Bass:
    global _NC_CACHE
    if _NC_CACHE is None:
        _NC_CACHE = build_nc()
        _NC_CACHE.compile()
    return _NC_CACHE


def run_spmd(x: np.ndarray, window: np.ndarray, **kwargs):
    """Shard, run on 8 cores, return (stacked output, BassKernelResults)."""
    global _CONST_CACHE
    if _CONST_CACHE is None or _CONST_CACHE[0] != window.tobytes():
        _CONST_CACHE = (window.tobytes(), make_consts(window))
    w1r, w2nr, d4 = _CONST_CACHE[1]
    in_maps = [
        {"x": np.ascontiguousarray(x[b].reshape(R, CN)),
         "w1r": w1r, "w2nr": w2nr, "d4": d4}
        for b in range(B)
    ]
    res = run_bass_kernel_spmd(nc=_get_nc(), in_maps=in_maps,
                               core_ids=list(range(B)), **kwargs)
    out = np.stack([res.results[b]["out"] for b in range(B)], axis=0)
    return out, res


def kernel(x: np.ndarray, window: np.ndarray) -> np.ndarray:
    out, _ = run_spmd(np.asarray(x), np.asarray(window))
    return out


# revision 31
# speedup vs baseline: 1.0835x; 1.0142x over previous
"""MDCT kernel for Trainium2 (8 NeuronCores, batch-parallel), folded DCT-IV form.

Math: frame f (hop N=1024, len 2N, center-padded) folds to an N-vector u and
out[f] = DCT-IV(u).  With x2 = x.reshape(1024, 1024) and y1 = w[:N]*x2[r],
y2 = w[N:]*x2[r] (per-row windowing):
    u[f, m]      = -y2[f, 511-m] - y2[f, 512+m]      (m < 512,  row f)
    u[f, 512+p]  =  y1[f-1, p]   - y1[f-1, 1023-p]   (p < 512,  row f-1)
so each x2 row r yields uLo[r] (frame r) and uHi[r] (frame r+1), and
    out[f, k] = sum_m u[f, m] * D4[m, k],   D4 = sqrt(2/N) DCT-IV matrix.

This halves the matmul contraction (1024 vs 2048) vs the direct form.  The
fold runs on the vector engine (reversals are negative-stride APs), u is
transposed on the PE in bf16 (1 cyc/row), and the DCT matmuls run in bf16
(same PE rate as fp32r, but half the weight-matrix HBM traffic).

Schedule notes (measured on HW):
- warm PE: MM N=512 ~216 ns, transpose ~56-107 ns; a run of dummy ident
  transposes spans the DMA-fill phase so the HAM clock gate is at 2.4 GHz
  when real work starts (a >3.4 us PE idle re-throttles to 1.2 GHz).
- emission is software-pipelined (fold r+2 / transpose r+1 / DCT r) so no
  engine queue blocks on a consumer stage; PSUM->SBUF u copies split
  between DVE (uLo) and ACT (uHi), out copies on ACT.
- DMA issue order: x0, w, x1, D0-3, x2, D4-7, x3..x7 - the fill phase is
  HBM-bandwidth-bound and DCT tile 0 needs all 8 D chunks early, while
  folds for rows 3+ happen well after their x tiles land.
- dma_start_transpose (XBAR) was 1.24 us per 128x128 call on the Sync
  sequencer (79 us total) - don't use it here; PE transposes are ~20x
  cheaper in stream.
"""

import numpy as np
import ml_dtypes

import concourse.bass as bass
import concourse.bacc as bacc
import concourse.mybir as mybir
import concourse.tile as tile
from concourse import masks
from concourse.bass_utils import run_bass_kernel_spmd

B = 8
T = 1 << 20
R = 1024          # rows of x2 per channel (T // hop)
CN = 1024         # row width (hop) = N
NF = 1025         # output frames
NK = 1024         # output bins
F32 = mybir.dt.float32
BF16 = mybir.dt.bfloat16

_NC_CACHE = None
_CONST_CACHE = None


def build_nc() -> bass.Bass:
    nc = bacc.Bacc("TRN2", target_bir_lowering=False, debug=False)
    x = nc.dram_tensor("x", [R, CN], F32, kind="ExternalInput").ap()
    w1r = nc.dram_tensor("w1r", [128, CN], BF16, kind="ExternalInput").ap()
    w2nr = nc.dram_tensor("w2nr", [128, CN], BF16, kind="ExternalInput").ap()
    d4 = nc.dram_tensor("d4", [8, 128, NK], BF16, kind="ExternalInput").ap()
    out = nc.dram_tensor("out", [NF, NK], F32, kind="ExternalOutput").ap()

    with tile.TileContext(nc) as tc:
        with (
            tc.tile_pool(name="persist", bufs=1) as persist,
            tc.tile_pool(name="xin", bufs=1) as xin,
            tc.tile_pool(name="ypool", bufs=6) as ypool,
            tc.tile_pool(name="upool", bufs=4) as upool,
            tc.tile_pool(name="outp", bufs=4) as outp,
            tc.tile_pool(name="tps", bufs=4, space="PSUM") as tps,
            tc.tile_pool(name="mmps", bufs=4, space="PSUM") as mmps,
        ):
            w1 = persist.tile([128, CN], BF16)
            w2n = persist.tile([128, CN], BF16)

            ident = persist.tile([128, 128], BF16)
            masks.make_identity(nc, ident[:])

            dt = persist.tile([128, 8, NK], BF16)
            ulot = persist.tile([128, 4, R], BF16)
            uhit = persist.tile([128, 4, NF], BF16)
            nc.vector.memset(uhit[:, :, 0:1], 0.0)

            xts = [xin.tile([128, CN], F32, name=f"xt{i}") for i in range(8)]

            def dma_x(r):
                nc.sync.dma_start(xts[r][:], x[r * 128:(r + 1) * 128, :])

            dma_x(0)
            nc.sync.dma_start(w2n[:], w2nr)
            nc.sync.dma_start(w1[:], w1r)
            dma_x(1)
            for ci in (0, 1, 2, 3):
                nc.sync.dma_start(dt[:, ci, :], d4[ci])
            dma_x(2)
            for ci in (4, 5, 6, 7):
                nc.sync.dma_start(dt[:, ci, :], d4[ci])
            for r in range(3, 8):
                dma_x(r)

            warm = tps.tile([128, 512], BF16, tag="tp")
            for _ in range(100):
                nc.tensor.transpose(warm[:, 0:128], ident[:], ident[:])

            uns = [None] * 8

            def fold(r: int):
                xt = xts[r]
                un = upool.tile([128, CN], BF16)
                r0 = r * 128
                y2n = ypool.tile([128, CN], BF16, tag="y2n")
                nc.vector.tensor_tensor(y2n[:], xt[:], w2n[:], mybir.AluOpType.mult)
                # uLo[m] = y2n[511-m] + y2n[512+m]   (y2n = -w2*x)
                nc.vector.tensor_tensor(
                    un[:, 0:512], y2n[:, 511::-1], y2n[:, 512:1024],
                    mybir.AluOpType.add,
                )
                plo = tps.tile([128, 512], BF16, tag="tp")
                for ci in range(4):
                    nc.tensor.transpose(
                        plo[:, ci * 128:(ci + 1) * 128],
                        un[:, ci * 128:(ci + 1) * 128], ident[:],
                    )
                nc.vector.tensor_copy(ulot[:, 0:4, r0:r0 + 128], plo[:])
                y1 = ypool.tile([128, CN], BF16, tag="y1")
                nc.vector.tensor_tensor(y1[:], xt[:], w1[:], mybir.AluOpType.mult)
                # uHi[p] = y1[p] - y1[1023-p]
                nc.vector.tensor_tensor(
                    un[:, 512:1024], y1[:, 0:512], y1[:, 1023:511:-1],
                    mybir.AluOpType.subtract,
                )
                phi = tps.tile([128, 512], BF16, tag="tp")
                for ci in range(4):
                    nc.tensor.transpose(
                        phi[:, ci * 128:(ci + 1) * 128],
                        un[:, 512 + ci * 128:512 + (ci + 1) * 128], ident[:],
                    )
                nc.scalar.copy(uhit[:, 0:4, 1 + r0:1 + r0 + 128], phi[:])
                uns[r] = un

            def transpose_u(r: int):
                pass

            def wslice(ci, f0):
                if ci < 4:
                    return ulot[:, ci, f0:f0 + 128]
                return uhit[:, ci - 4, f0:f0 + 128]

            def last_frame():
                # f=1024: only the uHi half (row 1023) contributes.
                pa = mmps.tile([1, 512], F32, tag="mm")
                pb = mmps.tile([1, 512], F32, tag="mm")
                for ci in range(4):
                    wsl = uhit[:, ci, 1024:1025]
                    nc.tensor.matmul(
                        pa[:], wsl, dt[:, 4 + ci, 0:512],
                        start=(ci == 0), stop=(ci == 3),
                    )
                    nc.tensor.matmul(
                        pb[:], wsl, dt[:, 4 + ci, 512:1024],
                        start=(ci == 0), stop=(ci == 3),
                    )
                ot = outp.tile([1, NK], F32, tag="ot_last")
                nc.scalar.copy(ot[:, 0:512], pa[:])
                nc.scalar.copy(ot[:, 512:1024], pb[:])
                nc.sync.dma_start(out[1024:1025, :], ot[:])

            def dct_tile(j: int):
                f0 = j * 128
                ot = outp.tile([128, NK], F32)
                pa = mmps.tile([128, 512], F32, tag="mm")
                for ci in range(8):
                    nc.tensor.matmul(
                        pa[:], wslice(ci, f0), dt[:, ci, 0:512],
                        start=(ci == 0), stop=(ci == 7),
                    )
                nc.scalar.copy(ot[:, 0:512], pa[:])
                nc.sync.dma_start(out[f0:f0 + 128, 0:512], ot[:, 0:512])
                pb = mmps.tile([128, 512], F32, tag="mm")
                for ci in range(8):
                    nc.tensor.matmul(
                        pb[:], wslice(ci, f0), dt[:, ci, 512:1024],
                        start=(ci == 0), stop=(ci == 7),
                    )
                nc.scalar.copy(ot[:, 512:768], pb[:, 0:256])
                nc.sync.dma_start(out[f0:f0 + 128, 512:768], ot[:, 512:768])
                nc.scalar.copy(ot[:, 768:1024], pb[:, 256:512])
                nc.sync.dma_start(out[f0:f0 + 128, 768:1024], ot[:, 768:1024])

            fold(0)
            transpose_u(0)
            fold(1)
            for r in range(8):
                if r + 2 < 8:
                    fold(r + 2)
                if r + 1 < 8:
                    transpose_u(r + 1)
                if r == 7:
                    last_frame()
                dct_tile(r)

    return nc


def make_consts(window: np.ndarray):
    w = window.astype(np.float64)
    w1r = np.broadcast_to(w[:CN].astype(ml_dtypes.bfloat16), (128, CN)).copy()
    w2nr = np.broadcast_to((-w[CN:]).astype(ml_dtypes.bfloat16), (128, CN)).copy()
    m = np.arange(NK, dtype=np.float64)[:, None]
    k = np.arange(NK, dtype=np.float64)[None, :]
    d = (np.sqrt(2.0 / NK) * np.cos(np.pi / NK * (m + 0.5) * (k + 0.5)))
    d4 = d.astype(ml_dtypes.bfloat16).reshape(8, 128, NK)
    return w1r, w2nr, d4


def _get_nc() -> bass.Bass:
    global _NC_CACHE
    if _NC_CACHE is None:
        _NC_CACHE = build_nc()
        _NC_CACHE.compile()
    return _NC_CACHE


def run_spmd(x: np.ndarray, window: np.ndarray, **kwargs):
    """Shard, run on 8 cores, return (stacked output, BassKernelResults)."""
    global _CONST_CACHE
    if _CONST_CACHE is None or _CONST_CACHE[0] != window.tobytes():
        _CONST_CACHE = (window.tobytes(), make_consts(window))
    w1r, w2nr, d4 = _CONST_CACHE[1]
    in_maps = [
        {"x": np.ascontiguousarray(x[b].reshape(R, CN)),
         "w1r": w1r, "w2nr": w2nr, "d4": d4}
        for b in range(B)
    ]
    res = run_bass_kernel_spmd(nc=_get_nc(), in_maps=in_maps,
                               core_ids=list(range(B)), **kwargs)
    out = np.stack([res.results[b]["out"] for b in range(B)], axis=0)
    return out, res


def kernel(x: np.ndarray, window: np.ndarray) -> np.ndarray:
    out, _ = run_spmd(np.asarray(x), np.asarray(window))
    return out


# revision 32
# speedup vs baseline: 1.0875x; 1.0037x over previous
"""MDCT kernel for Trainium2 (8 NeuronCores, batch-parallel), folded DCT-IV form.

Math: frame f (hop N=1024, len 2N, center-padded) folds to an N-vector u and
out[f] = DCT-IV(u).  With x2 = x.reshape(1024, 1024) and y1 = w[:N]*x2[r],
y2 = w[N:]*x2[r] (per-row windowing):
    u[f, m]      = -y2[f, 511-m] - y2[f, 512+m]      (m < 512,  row f)
    u[f, 512+p]  =  y1[f-1, p]   - y1[f-1, 1023-p]   (p < 512,  row f-1)
so each x2 row r yields uLo[r] (frame r) and uHi[r] (frame r+1), and
    out[f, k] = sum_m u[f, m] * D4[m, k],   D4 = sqrt(2/N) DCT-IV matrix.

This halves the matmul contraction (1024 vs 2048) vs the direct form.  The
fold runs on the vector engine (reversals are negative-stride APs), u is
transposed on the PE in bf16 (1 cyc/row), and the DCT matmuls run in bf16
(same PE rate as fp32r, but half the weight-matrix HBM traffic).

Schedule notes (measured on HW):
- warm PE: MM N=512 ~216 ns, transpose ~56-107 ns; a run of dummy ident
  transposes spans the DMA-fill phase so the HAM clock gate is at 2.4 GHz
  when real work starts (a >3.4 us PE idle re-throttles to 1.2 GHz).
- emission is software-pipelined (fold r+2 / transpose r+1 / DCT r) so no
  engine queue blocks on a consumer stage; PSUM->SBUF u copies split
  between DVE (uLo) and ACT (uHi), out copies on ACT.
- DMA issue order: x0, w, x1, D0-3, x2, D4-7, x3..x7 - the fill phase is
  HBM-bandwidth-bound and DCT tile 0 needs all 8 D chunks early, while
  folds for rows 3+ happen well after their x tiles land.
- dma_start_transpose (XBAR) was 1.24 us per 128x128 call on the Sync
  sequencer (79 us total) - don't use it here; PE transposes are ~20x
  cheaper in stream.
"""

import numpy as np
import ml_dtypes

import concourse.bass as bass
import concourse.bacc as bacc
import concourse.mybir as mybir
import concourse.tile as tile
from concourse import masks
from concourse.bass_utils import run_bass_kernel_spmd

B = 8
T = 1 << 20
R = 1024          # rows of x2 per channel (T // hop)
CN = 1024         # row width (hop) = N
NF = 1025         # output frames
NK = 1024         # output bins
F32 = mybir.dt.float32
BF16 = mybir.dt.bfloat16

_NC_CACHE = None
_CONST_CACHE = None


def build_nc() -> bass.Bass:
    nc = bacc.Bacc("TRN2", target_bir_lowering=False, debug=False)
    x = nc.dram_tensor("x", [R, CN], F32, kind="ExternalInput").ap()
    w1r = nc.dram_tensor("w1r", [128, CN], BF16, kind="ExternalInput").ap()
    w2nr = nc.dram_tensor("w2nr", [128, CN], BF16, kind="ExternalInput").ap()
    d4 = nc.dram_tensor("d4", [8, 128, NK], BF16, kind="ExternalInput").ap()
    out = nc.dram_tensor("out", [NF, NK], F32, kind="ExternalOutput").ap()

    with tile.TileContext(nc) as tc:
        with (
            tc.tile_pool(name="persist", bufs=1) as persist,
            tc.tile_pool(name="xin", bufs=1) as xin,
            tc.tile_pool(name="ypool", bufs=6) as ypool,
            tc.tile_pool(name="upool", bufs=4) as upool,
            tc.tile_pool(name="outp", bufs=4) as outp,
            tc.tile_pool(name="tps", bufs=4, space="PSUM") as tps,
            tc.tile_pool(name="mmps", bufs=4, space="PSUM") as mmps,
        ):
            w1 = persist.tile([128, CN], BF16)
            w2n = persist.tile([128, CN], BF16)

            ident = persist.tile([128, 128], BF16)
            masks.make_identity(nc, ident[:])

            dt = persist.tile([128, 8, NK], BF16)
            ulot = persist.tile([128, 4, R], BF16)
            uhit = persist.tile([128, 4, NF], BF16)
            nc.vector.memset(uhit[:, :, 0:1], 0.0)

            xts = [xin.tile([128, CN], F32, name=f"xt{i}") for i in range(8)]

            def dma_x(r):
                nc.sync.dma_start(xts[r][:], x[r * 128:(r + 1) * 128, :])

            dma_x(0)
            nc.sync.dma_start(w2n[:], w2nr)
            nc.sync.dma_start(w1[:], w1r)
            dma_x(1)
            for ci in (0, 1, 2, 3):
                nc.sync.dma_start(dt[:, ci, :], d4[ci])
            dma_x(2)
            for ci in (4, 5, 6, 7):
                nc.sync.dma_start(dt[:, ci, :], d4[ci])
            for r in range(3, 8):
                dma_x(r)

            warm = tps.tile([128, 512], BF16, tag="tp")
            for _ in range(100):
                nc.tensor.transpose(warm[:, 0:128], ident[:], ident[:])

            uns = [None] * 8

            def fold(r: int):
                xt = xts[r]
                un = upool.tile([128, CN], BF16)
                r0 = r * 128
                y2n = ypool.tile([128, CN], BF16, tag="y2n")
                nc.vector.tensor_tensor(y2n[:], xt[:], w2n[:], mybir.AluOpType.mult)
                # uLo[m] = y2n[511-m] + y2n[512+m]   (y2n = -w2*x)
                nc.vector.tensor_tensor(
                    un[:, 0:512], y2n[:, 511::-1], y2n[:, 512:1024],
                    mybir.AluOpType.add,
                )
                plo = tps.tile([128, 512], BF16, tag="tp")
                for ci in range(4):
                    nc.tensor.transpose(
                        plo[:, ci * 128:(ci + 1) * 128],
                        un[:, ci * 128:(ci + 1) * 128], ident[:],
                    )
                nc.vector.tensor_copy(ulot[:, 0:4, r0:r0 + 128], plo[:])
                y1 = ypool.tile([128, CN], BF16, tag="y1")
                nc.vector.tensor_tensor(y1[:], xt[:], w1[:], mybir.AluOpType.mult)
                # uHi[p] = y1[p] - y1[1023-p]
                nc.vector.tensor_tensor(
                    un[:, 512:1024], y1[:, 0:512], y1[:, 1023:511:-1],
                    mybir.AluOpType.subtract,
                )
                phi = tps.tile([128, 512], BF16, tag="tp")
                for ci in range(4):
                    nc.tensor.transpose(
                        phi[:, ci * 128:(ci + 1) * 128],
                        un[:, 512 + ci * 128:512 + (ci + 1) * 128], ident[:],
                    )
                nc.scalar.copy(uhit[:, 0:4, 1 + r0:1 + r0 + 128], phi[:])
                uns[r] = un

            def transpose_u(r: int):
                pass

            def wslice(ci, f0):
                if ci < 4:
                    return ulot[:, ci, f0:f0 + 128]
                return uhit[:, ci - 4, f0:f0 + 128]

            def last_frame():
                # f=1024: only the uHi half (row 1023) contributes.
                pa = mmps.tile([1, 512], F32, tag="mm")
                pb = mmps.tile([1, 512], F32, tag="mm")
                for ci in range(4):
                    wsl = uhit[:, ci, 1024:1025]
                    nc.tensor.matmul(
                        pa[:], wsl, dt[:, 4 + ci, 0:512],
                        start=(ci == 0), stop=(ci == 3),
                    )
                    nc.tensor.matmul(
                        pb[:], wsl, dt[:, 4 + ci, 512:1024],
                        start=(ci == 0), stop=(ci == 3),
                    )
                ot = outp.tile([1, NK], F32, tag="ot_last")
                nc.scalar.copy(ot[:, 0:512], pa[:])
                nc.vector.tensor_copy(ot[:, 512:1024], pb[:])
                nc.sync.dma_start(out[1024:1025, :], ot[:])

            def dct_tile(j: int):
                f0 = j * 128
                ot = outp.tile([128, NK], F32)
                pa = mmps.tile([128, 512], F32, tag="mm")
                for ci in range(8):
                    nc.tensor.matmul(
                        pa[:], wslice(ci, f0), dt[:, ci, 0:512],
                        start=(ci == 0), stop=(ci == 7),
                    )
                nc.scalar.copy(ot[:, 0:512], pa[:])
                nc.sync.dma_start(out[f0:f0 + 128, 0:512], ot[:, 0:512])
                pb = mmps.tile([128, 512], F32, tag="mm")
                for ci in range(8):
                    nc.tensor.matmul(
                        pb[:], wslice(ci, f0), dt[:, ci, 512:1024],
                        start=(ci == 0), stop=(ci == 7),
                    )
                nc.scalar.copy(ot[:, 512:768], pb[:, 0:256])
                nc.sync.dma_start(out[f0:f0 + 128, 512:768], ot[:, 512:768])
                nc.vector.tensor_copy(ot[:, 768:1024], pb[:, 256:512])
                nc.sync.dma_start(out[f0:f0 + 128, 768:1024], ot[:, 768:1024])

            fold(0)
            transpose_u(0)
            fold(1)
            for r in range(8):
                if r + 2 < 8:
                    fold(r + 2)
                if r + 1 < 8:
                    transpose_u(r + 1)
                if r == 7:
                    last_frame()
                dct_tile(r)

    return nc


def make_consts(window: np.ndarray):
    w = window.astype(np.float64)
    w1r = np.broadcast_to(w[:CN].astype(ml_dtypes.bfloat16), (128, CN)).copy()
    w2nr = np.broadcast_to((-w[CN:]).astype(ml_dtypes.bfloat16), (128, CN)).copy()
    m = np.arange(NK, dtype=np.float64)[:, None]
    k = np.arange(NK, dtype=np.float64)[None, :]
    d = (np.sqrt(2.0 / NK) * np.cos(np.pi / NK * (m + 0.5) * (k + 0.5)))
    d4 = d.astype(ml_dtypes.bfloat16).reshape(8, 128, NK)
    return w1r, w2nr, d4


def _get_nc() -> bass.Bass:
    global _NC_CACHE
    if _NC_CACHE is None:
        _NC_CACHE = build_nc()
        _NC_CACHE.compile()
    return _NC_CACHE


def run_spmd(x: np.ndarray, window: np.ndarray, **kwargs):
    """Shard, run on 8 cores, return (stacked output, BassKernelResults)."""
    global _CONST_CACHE
    if _CONST_CACHE is None or _CONST_CACHE[0] != window.tobytes():
        _CONST_CACHE = (window.tobytes(), make_consts(window))
    w1r, w2nr, d4 = _CONST_CACHE[1]
    in_maps = [
        {"x": np.ascontiguousarray(x[b].reshape(R, CN)),
         "w1r": w1r, "w2nr": w2nr, "d4": d4}
        for b in range(B)
    ]
    res = run_bass_kernel_spmd(nc=_get_nc(), in_maps=in_maps,
                               core_ids=list(range(B)), **kwargs)
    out = np.stack([res.results[b]["out"] for b in range(B)], axis=0)
    return out, res


def kernel(x: np.ndarray, window: np.ndarray) -> np.ndarray:
    out, _ = run_spmd(np.asarray(x), np.asarray(window))
    return out


# revision 33
# speedup vs baseline: 1.0899x; 1.0022x over previous
"""MDCT kernel for Trainium2 (8 NeuronCores, batch-parallel), folded DCT-IV form.

Math: frame f (hop N=1024, len 2N, center-padded) folds to an N-vector u and
out[f] = DCT-IV(u).  With x2 = x.reshape(1024, 1024) and y1 = w[:N]*x2[r],
y2 = w[N:]*x2[r] (per-row windowing):
    u[f, m]      = -y2[f, 511-m] - y2[f, 512+m]      (m < 512,  row f)
    u[f, 512+p]  =  y1[f-1, p]   - y1[f-1, 1023-p]   (p < 512,  row f-1)
so each x2 row r yields uLo[r] (frame r) and uHi[r] (frame r+1), and
    out[f, k] = sum_m u[f, m] * D4[m, k],   D4 = sqrt(2/N) DCT-IV matrix.

This halves the matmul contraction (1024 vs 2048) vs the direct form.  The
fold runs on the vector engine (reversals are negative-stride APs), u is
transposed on the PE in bf16 (1 cyc/row), and the DCT matmuls run in bf16
(same PE rate as fp32r, but half the weight-matrix HBM traffic).

Schedule notes (measured on HW):
- warm PE: MM N=512 ~216 ns, transpose ~56-107 ns; a run of dummy ident
  transposes spans the DMA-fill phase so the HAM clock gate is at 2.4 GHz
  when real work starts (a >3.4 us PE idle re-throttles to 1.2 GHz).
- emission is software-pipelined (fold r+2 / transpose r+1 / DCT r) so no
  engine queue blocks on a consumer stage; PSUM->SBUF u copies split
  between DVE (uLo) and ACT (uHi), out copies on ACT.
- DMA issue order: x0, w, x1, D0-3, x2, D4-7, x3..x7 - the fill phase is
  HBM-bandwidth-bound and DCT tile 0 needs all 8 D chunks early, while
  folds for rows 3+ happen well after their x tiles land.
- dma_start_transpose (XBAR) was 1.24 us per 128x128 call on the Sync
  sequencer (79 us total) - don't use it here; PE transposes are ~20x
  cheaper in stream.
"""

import numpy as np
import ml_dtypes

import concourse.bass as bass
import concourse.bacc as bacc
import concourse.mybir as mybir
import concourse.tile as tile
from concourse import masks
from concourse.bass_utils import run_bass_kernel_spmd

B = 8
T = 1 << 20
R = 1024          # rows of x2 per channel (T // hop)
CN = 1024         # row width (hop) = N
NF = 1025         # output frames
NK = 1024         # output bins
F32 = mybir.dt.float32
BF16 = mybir.dt.bfloat16

_NC_CACHE = None
_CONST_CACHE = None


def build_nc() -> bass.Bass:
    nc = bacc.Bacc("TRN2", target_bir_lowering=False, debug=False)
    x = nc.dram_tensor("x", [R, CN], F32, kind="ExternalInput").ap()
    w1r = nc.dram_tensor("w1r", [128, CN], BF16, kind="ExternalInput").ap()
    w2nr = nc.dram_tensor("w2nr", [128, CN], BF16, kind="ExternalInput").ap()
    d4 = nc.dram_tensor("d4", [8, 128, NK], BF16, kind="ExternalInput").ap()
    out = nc.dram_tensor("out", [NF, NK], F32, kind="ExternalOutput").ap()

    with tile.TileContext(nc) as tc:
        with (
            tc.tile_pool(name="persist", bufs=1) as persist,
            tc.tile_pool(name="xin", bufs=1) as xin,
            tc.tile_pool(name="ypool", bufs=6) as ypool,
            tc.tile_pool(name="upool", bufs=4) as upool,
            tc.tile_pool(name="outp", bufs=4) as outp,
            tc.tile_pool(name="tps", bufs=2, space="PSUM") as tps,
            tc.tile_pool(name="mmps", bufs=6, space="PSUM") as mmps,
        ):
            w1 = persist.tile([128, CN], BF16)
            w2n = persist.tile([128, CN], BF16)

            ident = persist.tile([128, 128], BF16)
            masks.make_identity(nc, ident[:])

            dt = persist.tile([128, 8, NK], BF16)
            ulot = persist.tile([128, 4, R], BF16)
            uhit = persist.tile([128, 4, NF], BF16)
            nc.vector.memset(uhit[:, :, 0:1], 0.0)

            xts = [xin.tile([128, CN], F32, name=f"xt{i}") for i in range(8)]

            def dma_x(r):
                nc.sync.dma_start(xts[r][:], x[r * 128:(r + 1) * 128, :])

            dma_x(0)
            nc.sync.dma_start(w2n[:], w2nr)
            nc.sync.dma_start(w1[:], w1r)
            dma_x(1)
            for ci in (0, 1, 2, 3):
                nc.sync.dma_start(dt[:, ci, :], d4[ci])
            dma_x(2)
            for ci in (4, 5, 6, 7):
                nc.sync.dma_start(dt[:, ci, :], d4[ci])
            for r in range(3, 8):
                dma_x(r)

            warm = tps.tile([128, 512], BF16, tag="tp")
            for _ in range(100):
                nc.tensor.transpose(warm[:, 0:128], ident[:], ident[:])

            uns = [None] * 8

            def fold(r: int):
                xt = xts[r]
                un = upool.tile([128, CN], BF16)
                r0 = r * 128
                y2n = ypool.tile([128, CN], BF16, tag="y2n")
                nc.vector.tensor_tensor(y2n[:], xt[:], w2n[:], mybir.AluOpType.mult)
                # uLo[m] = y2n[511-m] + y2n[512+m]   (y2n = -w2*x)
                nc.vector.tensor_tensor(
                    un[:, 0:512], y2n[:, 511::-1], y2n[:, 512:1024],
                    mybir.AluOpType.add,
                )
                plo = tps.tile([128, 512], BF16, tag="tp")
                for ci in range(4):
                    nc.tensor.transpose(
                        plo[:, ci * 128:(ci + 1) * 128],
                        un[:, ci * 128:(ci + 1) * 128], ident[:],
                    )
                nc.vector.tensor_copy(ulot[:, 0:4, r0:r0 + 128], plo[:])
                y1 = ypool.tile([128, CN], BF16, tag="y1")
                nc.vector.tensor_tensor(y1[:], xt[:], w1[:], mybir.AluOpType.mult)
                # uHi[p] = y1[p] - y1[1023-p]
                nc.vector.tensor_tensor(
                    un[:, 512:1024], y1[:, 0:512], y1[:, 1023:511:-1],
                    mybir.AluOpType.subtract,
                )
                phi = tps.tile([128, 512], BF16, tag="tp")
                for ci in range(4):
                    nc.tensor.transpose(
                        phi[:, ci * 128:(ci + 1) * 128],
                        un[:, 512 + ci * 128:512 + (ci + 1) * 128], ident[:],
                    )
                nc.scalar.copy(uhit[:, 0:4, 1 + r0:1 + r0 + 128], phi[:])
                uns[r] = un

            def transpose_u(r: int):
                pass

            def wslice(ci, f0):
                if ci < 4:
                    return ulot[:, ci, f0:f0 + 128]
                return uhit[:, ci - 4, f0:f0 + 128]

            def last_frame():
                # f=1024: only the uHi half (row 1023) contributes.
                pa = mmps.tile([1, 512], F32, tag="mm")
                pb = mmps.tile([1, 512], F32, tag="mm")
                for ci in range(4):
                    wsl = uhit[:, ci, 1024:1025]
                    nc.tensor.matmul(
                        pa[:], wsl, dt[:, 4 + ci, 0:512],
                        start=(ci == 0), stop=(ci == 3),
                    )
                    nc.tensor.matmul(
                        pb[:], wsl, dt[:, 4 + ci, 512:1024],
                        start=(ci == 0), stop=(ci == 3),
                    )
                ot = outp.tile([1, NK], F32, tag="ot_last")
                nc.scalar.copy(ot[:, 0:512], pa[:])
                nc.vector.tensor_copy(ot[:, 512:1024], pb[:])
                nc.sync.dma_start(out[1024:1025, :], ot[:])

            def dct_tile(j: int):
                f0 = j * 128
                ot = outp.tile([128, NK], F32)
                pa = mmps.tile([128, 512], F32, tag="mm")
                for ci in range(8):
                    nc.tensor.matmul(
                        pa[:], wslice(ci, f0), dt[:, ci, 0:512],
                        start=(ci == 0), stop=(ci == 7),
                    )
                nc.scalar.copy(ot[:, 0:512], pa[:])
                nc.sync.dma_start(out[f0:f0 + 128, 0:512], ot[:, 0:512])
                pb = mmps.tile([128, 512], F32, tag="mm")
                for ci in range(8):
                    nc.tensor.matmul(
                        pb[:], wslice(ci, f0), dt[:, ci, 512:1024],
                        start=(ci == 0), stop=(ci == 7),
                    )
                nc.scalar.copy(ot[:, 512:768], pb[:, 0:256])
                nc.sync.dma_start(out[f0:f0 + 128, 512:768], ot[:, 512:768])
                nc.vector.tensor_copy(ot[:, 768:1024], pb[:, 256:512])
                nc.sync.dma_start(out[f0:f0 + 128, 768:1024], ot[:, 768:1024])

            fold(0)
            transpose_u(0)
            fold(1)
            for r in range(8):
                if r + 2 < 8:
                    fold(r + 2)
                if r + 1 < 8:
                    transpose_u(r + 1)
                if r == 7:
                    last_frame()
                dct_tile(r)

    return nc


def make_consts(window: np.ndarray):
    w = window.astype(np.float64)
    w1r = np.broadcast_to(w[:CN].astype(ml_dtypes.bfloat16), (128, CN)).copy()
    w2nr = np.broadcast_to((-w[CN:]).astype(ml_dtypes.bfloat16), (128, CN)).copy()
    m = np.arange(NK, dtype=np.float64)[:, None]
    k = np.arange(NK, dtype=np.float64)[None, :]
    d = (np.sqrt(2.0 / NK) * np.cos(np.pi / NK * (m + 0.5) * (k + 0.5)))
    d4 = d.astype(ml_dtypes.bfloat16).reshape(8, 128, NK)
    return w1r, w2nr, d4


def _get_nc() -> bass.Bass:
    global _NC_CACHE
    if _NC_CACHE is None:
        _NC_CACHE = build_nc()
        _NC_CACHE.compile()
    return _NC_CACHE


def run_spmd(x: np.ndarray, window: np.ndarray, **kwargs):
    """Shard, run on 8 cores, return (stacked output, BassKernelResults)."""
    global _CONST_CACHE
    if _CONST_CACHE is None or _CONST_CACHE[0] != window.tobytes():
        _CONST_CACHE = (window.tobytes(), make_consts(window))
    w1r, w2nr, d4 = _CONST_CACHE[1]
    in_maps = [
        {"x": np.ascontiguousarray(x[b].reshape(R, CN)),
         "w1r": w1r, "w2nr": w2nr, "d4": d4}
        for b in range(B)
    ]
    res = run_bass_kernel_spmd(nc=_get_nc(), in_maps=in_maps,
                               core_ids=list(range(B)), **kwargs)
    out = np.stack([res.results[b]["out"] for b in range(B)], axis=0)
    return out, res


def kernel(x: np.ndarray, window: np.ndarray) -> np.ndarray:
    out, _ = run_spmd(np.asarray(x), np.asarray(window))
    return out
